# revision 13
# baseline (speedup 1.0000x reference)
"""Distributed Bass/Tile kernel for a dense transformer block on 8 TRN2 NeuronCores.

Sharding: sequence-parallel. Flattened tokens [B*S] are split into 8 chunks of
TOK=512 tokens; cores 0-3 hold batch 0, cores 4-7 batch 1. Each core computes
LN1 -> QKV for its chunk, AllGathers K^T and V (groups of 4 = one batch),
runs full attention for its query chunk, then proj+residual, LN2, and the FFN
row-parallel with replicated weights. No all-reduce is needed.

Scheduling notes:
- K^T/V/Q/exp-scores are fp8e4m3: softmax averaging over a diffuse A makes
  the quantization noise negligible, and it halves the gather bytes. The FFN
  stays bf16 (fp8 there costs ~2e-2 rel err). Residual stream is f32.
- LN1 is folded into QKV: xgb = g1*x is built as x streams in, the -mean
  rank-1 term rides each QKV psum chain as a 1-row outer-product matmul, and
  rstd is applied in the per-block epilogue. No normalize wall: the first K
  gather launches as soon as 4 K blocks + the LN1 rows exist (~25us).
- HWDGE DMA triggers block the issuing engine until the DMA is admitted, so
  collective-gated loads (gather-ins) go on the SP queue together with
  bounce writes / norm shifts / output; weight streams ride the ACT queue
  (idle outside attention) except proj's, which go on SP pre-attention.
- The K gather is split in two (heads 0-7 / 8-15) around the V gather.
- Attention: scores run 4 heads ahead of AV (36-deep fp8 e-pool), exp (the
  ACT-bound floor, ~140us) sits between; per-head normalize lags 5 heads.
- LN2 stats matmuls are interleaved into the proj loop; normalize folds g2
  into per-tile PE outer-product broadcasts (2 DVE ops per tile).
"""

import numpy as np
import ml_dtypes

import concourse.bacc as bacc
import concourse.mybir as mybir
import concourse.tile as tile
from concourse.bass_utils import run_bass_kernel_spmd

F32 = mybir.dt.float32
BF16 = mybir.dt.bfloat16
FP8 = mybir.dt.float8e4

FULL_DIMS = dict(E=1024, H=16, DH=64, TOK=512, G=4, NC=8, FF=4096)


def build_nc(dims):
    E, H, DH, TOK, G, NC, FF = (
        dims["E"], dims["H"], dims["DH"], dims["TOK"], dims["G"], dims["NC"], dims["FF"]
    )
    ET = E // 128
    FT = FF // 128
    TOKT = TOK // 128
    KT = G * TOKT
    NG = KT // 2
    HD1 = DH + 1
    HDH = H * DH
    NV = (HDH + 511) // 512
    HPN = 512 // DH
    eps = 1e-5
    sm_scale = float(DH) ** -0.5
    add, mult, mx = mybir.AluOpType.add, mybir.AluOpType.mult, mybir.AluOpType.max
    Exp = mybir.ActivationFunctionType.Exp

    groups = [list(range(g * G, (g + 1) * G)) for g in range(NC // G)]

    nc = bacc.Bacc("TRN2", target_bir_lowering=False, debug=False, num_devices=NC)

    def din(name, shape, dt=BF16):
        return nc.dram_tensor(name, shape, dt, kind="ExternalInput").ap()

    x_d = din("x", [128, ET * TOK], F32)
    wq_d = din("wq", [128, ET * ET * 128])
    wk_d = din("wk", [128, ET * ET * 128])
    wv_d = din("wv", [128, ET * E])           # row-major (moving operand)
    wproj_d = din("wproj", [128, ET * ET * 128])
    w1_d = din("w1", [128, FT * ET * 128])
    w2_d = din("w2", [128, ET * FT * 128])
    wqkvg_d = din("wqkvg", [1, 3 * HDH])      # W^T g1 for q,k,v
    wvbr_d = din("wvbr", [1, HDH])            # Wv^T beta1 row
    g1c_d = din("g1c", [128, ET], F32)
    g2r_d = din("g2r", [1, E], BF16)
    wqbc_d = din("wqbc", [128, ET], F32)      # Wq^T beta1 as columns
    wkbc_d = din("wkbc", [128, ET], F32)
    be2_d = din("be2c", [128, ET], F32)
    bproj_d = din("bprojc", [128, ET], F32)
    b1_d = din("b1c", [128, FT], F32)
    b2_d = din("b2c", [128, ET], F32)
    out_d = nc.dram_tensor("outT", [128, ET * TOK], F32, kind="ExternalOutput").ap()

    ones_col_bf = nc.const_aps.tensor(1.0, (128, 1), BF16)
    ones_col_f32 = nc.const_aps.tensor(1.0, (128, 1), F32)
    zeros_bc = nc.const_aps.tensor(0.0, (128, TOK), F32)

    KH = ET // 2 * TOK
    VW = TOKT * H * HD1
    kvslot = G * VW

    with tile.TileContext(nc) as tc:
        with (
            tc.tile_pool(name="dram", bufs=1, space="DRAM") as dram,
            tc.tile_pool(name="resid", bufs=2) as resid,
            tc.tile_pool(name="acts", bufs=2) as acts,
            tc.tile_pool(name="loc8", bufs=3) as loc8,
            tc.tile_pool(name="kv", bufs=2) as kvp,
            tc.tile_pool(name="small", bufs=1) as small,
            tc.tile_pool(name="wstr", bufs=3) as wstr,
            tc.tile_pool(name="wstr2", bufs=2) as wstr2,
            tc.tile_pool(name="rows", bufs=1) as rows,
            tc.tile_pool(name="rr", bufs=2) as rr,
            tc.tile_pool(name="scr", bufs=1) as scr,
            tc.tile_pool(name="expp", bufs=30) as expp,
            tc.tile_pool(name="shp", bufs=2) as shp,
        ):
            # ---- constant / input loads (SP queue except weights) ----
            x_sb = resid.tile([128, ET * TOK], F32, tag="resid")
            for k in range(ET):
                sl = slice(k * TOK, (k + 1) * TOK)
                nc.sync.dma_start(x_sb[:, sl], x_d[:, sl])
            cols = small.tile([128, 6 * ET + FT], F32, tag="cols")
            for i, d in enumerate([g1c_d, wqbc_d, wkbc_d, be2_d, bproj_d, b2_d]):
                nc.sync.dma_start(cols[:, i * ET:(i + 1) * ET], d)
            nc.sync.dma_start(cols[:, 6 * ET:6 * ET + FT], b1_d)
            g1c = cols[:, 0 * ET:1 * ET]
            wqbc = cols[:, 1 * ET:2 * ET]
            wkbc = cols[:, 2 * ET:3 * ET]
            be2c = cols[:, 3 * ET:4 * ET]
            bprojc = cols[:, 4 * ET:5 * ET]
            b2c = cols[:, 5 * ET:6 * ET]
            b1c = cols[:, 6 * ET:6 * ET + FT]
            grow = small.tile([1, E], BF16, tag="grow")
            nc.sync.dma_start(grow[:], g2r_d)
            wqkvg = small.tile([1, 3 * HDH], BF16, tag="wqkvg")
            nc.sync.dma_start(wqkvg[:], wqkvg_d)
            wvbr = small.tile([1, HDH], BF16, tag="wvbr")
            nc.sync.dma_start(wvbr[:], wvbr_d)
            ones_full = small.tile([128, 128], BF16, tag="ones")
            nc.vector.memset(ones_full[:], 1.0)
            wv_sb = kvp.tile([128, kvslot // 2], BF16, tag="kv")
            nc.scalar.dma_start(wv_sb[:, 0:ET * E], wv_d)

            # ================= LN1 stats + fold prep =================
            xgb = acts.tile([128, ET * TOK], BF16, tag="act8")
            with tc.tile_pool(name="lnps", bufs=1, space="PSUM") as lnps, \
                 tc.tile_pool(name="bc0", bufs=1, space="PSUM") as bc0:
                st_s = lnps.tile([1, TOK], F32, tag="st_s")
                st_q = lnps.tile([1, TOK], F32, tag="st_q")
                for k in range(ET):
                    sl = slice(k * TOK, (k + 1) * TOK)
                    sq = scr.tile([128, TOK], BF16, tag="lnsq", bufs=2)
                    nc.vector.tensor_mul(sq[:], x_sb[:, sl], x_sb[:, sl])
                    nc.vector.scalar_tensor_tensor(
                        out=xgb[:, sl], in0=x_sb[:, sl],
                        scalar=g1c[:, k:k + 1], op0=mult,
                        in1=zeros_bc, op1=add)
                    nc.tensor.matmul(st_s[:], ones_col_f32, x_sb[:, sl],
                                     start=(k == 0), stop=(k == ET - 1))
                    nc.tensor.matmul(st_q[:], ones_col_bf, sq[:],
                                     start=(k == 0), stop=(k == ET - 1))
                r_mean = rows.tile([1, TOK], F32, tag="rowf")
                r_m2 = rows.tile([1, TOK], F32, tag="rowf2")
                r_msq = rows.tile([1, TOK], F32, tag="rowf3")
                r_var = rows.tile([1, TOK], F32, tag="rowf2b")
                r_rec = rows.tile([1, TOK], F32, tag="rowf3b")
                r_rstd = rows.tile([1, TOK], F32, tag="rowf4")
                r_nmean = rows.tile([1, TOK], F32, tag="rowf5")
                nc.vector.tensor_scalar_mul(r_mean[:], st_s[:], 1.0 / E)
                nc.vector.tensor_scalar_mul(r_m2[:], st_q[:], 1.0 / E)
                nc.vector.tensor_mul(r_msq[:], r_mean[:], r_mean[:])
                nc.vector.tensor_sub(r_var[:], r_m2[:], r_msq[:])
                nc.vector.tensor_scalar_add(r_var[:], r_var[:], eps)
                nc.vector.reciprocal_approx_fast(r_rec[:], r_var[:])
                nc.scalar.sqrt(r_rstd[:], r_rec[:])
                nc.vector.tensor_scalar_mul(r_nmean[:], r_mean[:], -1.0)
                r_rstd_bf = rows.tile([1, TOK], BF16, tag="rowbf")
                r_nmean_bf = rows.tile([1, TOK], BF16, tag="rowbf2")
                nc.vector.tensor_copy(r_rstd_bf[:], r_rstd[:])
                nc.vector.tensor_copy(r_nmean_bf[:], r_nmean[:])
                # rstd broadcast to SBUF (for K/Q epilogues, free-dim tokens)
                ps_r = bc0.tile([128, TOK], F32, tag="bcr")
                nc.tensor.matmul(ps_r[:], ones_full[0:1, :], r_rstd_bf[:],
                                 start=True, stop=True)
                rstd_sb = small.tile([128, TOK], F32, tag="rstd_sb")
                nc.vector.tensor_copy(rstd_sb[:], ps_r[:])
                # rstd as per-partition columns (for V epilogue, token rows)
                rstd_col = small.tile([128, TOKT], F32, tag="rstd_col")
                for c in range(TOKT):
                    nc.sync.dma_start(rstd_col[:, c:c + 1],
                                      r_rstd[0:1, c * 128:(c + 1) * 128])
                # Wv^T beta1 broadcast to all partitions
                wvb_sb = small.tile([128, HDH], BF16, tag="wvb")
                for nn in range(NV):
                    w = min(512, HDH - nn * 512)
                    ps_b = bc0.tile([128, 512], F32, tag="bcb")
                    nc.tensor.matmul(
                        ps_b[:, 0:w], ones_full[0:1, :],
                        wvbr[0:1, nn * 512: nn * 512 + w],
                        start=True, stop=True)
                    nc.vector.tensor_copy(wvb_sb[:, nn * 512:nn * 512 + w],
                                          ps_b[:, 0:w])
            # pre-load the Exp table while the ACT engine is idle
            warm = rows.tile([1, 1], F32, tag="warm")
            nc.scalar.activation(warm[:], cols[0:1, 0:1], Exp)

            # ================= K / V / Q (LN1-folded) =================
            kb1 = dram.tile([128, KH], FP8, tag="kb1")
            kb2 = dram.tile([128, KH], FP8, tag="kb2")
            kall1 = dram.tile([G * 128, KH], FP8, tag="ka1")
            kall2 = dram.tile([G * 128, KH], FP8, tag="ka2")
            vb = dram.tile([128, VW], FP8, tag="vb")
            vall = dram.tile([G * 128, VW], FP8, tag="va")

            def qk_block(dst, m, w_d, gseg, bcol):
                """dst[:, m*TOK:...] = rstd * (W^T xgb + outer(Wg, -mean)) + b"""
                wblk = wstr.tile([128, ET * 128], BF16, tag="wa")
                nc.scalar.dma_start(wblk[:], w_d[:, m * ET * 128:(m + 1) * ET * 128])
                ps = qkvps.tile([128, TOK], F32, tag="mm")
                for k in range(ET):
                    nc.tensor.matmul(ps[:], wblk[:, k * 128:(k + 1) * 128],
                                     xgb[:, k * TOK:(k + 1) * TOK],
                                     start=(k == 0), stop=False)
                nc.tensor.matmul(ps[:],
                                 wqkvg[0:1, gseg + m * 128: gseg + (m + 1) * 128],
                                 r_nmean_bf[:], start=False, stop=True)
                t1 = scr.tile([128, TOK], F32, tag="qk_t1", bufs=2)
                nc.vector.tensor_mul(t1[:], ps[:], rstd_sb[:])
                nc.vector.scalar_tensor_tensor(
                    out=dst[:, m * TOK:(m + 1) * TOK], in0=t1[:],
                    scalar=bcol[:, m:m + 1], op0=add, in1=zeros_bc, op1=add)

            ktloc = loc8.tile([128, ET * TOK], FP8, tag="loc")
            with tc.tile_pool(name="qkvps", bufs=2, space="PSUM") as qkvps:
                for m in range(ET):
                    qk_block(ktloc, m, wk_d, HDH, wkbc)
                    if m == ET // 2 - 1:
                        nc.sync.dma_start(kb1[:], ktloc[:, 0:KH])
                        nc.gpsimd.collective_compute(
                            "AllGather", mybir.AluOpType.bypass, replica_groups=groups,
                            ins=[kb1.opt()], outs=[kall1.opt()])
                    if m == ET - 1:
                        nc.sync.dma_start(kb2[:], ktloc[:, KH:2 * KH])

                # ---- V ----
                vloc = loc8.tile([128, VW], FP8, tag="loc")
                vloc4 = vloc[:].rearrange("p (t h d) -> p t h d", t=TOKT, h=H, d=HD1)
                nc.vector.memset(vloc4[:, :, :, DH:DH + 1], 1.0)
                for tt in range(TOKT):
                    for nn in range(NV):
                        w = min(512, HDH - nn * 512)
                        ps = qkvps.tile([128, max(TOK, 512)], F32, tag="mm")
                        for k in range(ET):
                            nc.tensor.matmul(
                                ps[:, 0:w],
                                xgb[:, k * TOK + tt * 128: k * TOK + tt * 128 + 128],
                                wv_sb[:, k * E + nn * 512: k * E + nn * 512 + w],
                                start=(k == 0), stop=False)
                        nc.tensor.matmul(
                            ps[:, 0:w],
                            r_nmean_bf[0:1, tt * 128:(tt + 1) * 128],
                            wqkvg[0:1, 2 * HDH + nn * 512: 2 * HDH + nn * 512 + w],
                            start=False, stop=True)
                        nhd = w // DH
                        vt = scr.tile([128, 512], FP8, tag="vt", bufs=2)
                        nc.vector.scalar_tensor_tensor(
                            out=vt[:, 0:w], in0=ps[:, 0:w],
                            scalar=rstd_col[:, tt:tt + 1], op0=mult,
                            in1=wvb_sb[:, nn * 512:nn * 512 + w], op1=add)
                        src = vt[:, 0:w].rearrange("p (h d) -> p h d", h=nhd, d=DH)
                        dst = vloc4[:, tt:tt + 1, nn * HPN:nn * HPN + nhd, 0:DH]
                        nc.vector.tensor_copy(dst.opt(), src)
                nc.sync.dma_start(vb[:], vloc[:])
                nc.gpsimd.collective_compute(
                    "AllGather", mybir.AluOpType.bypass, replica_groups=groups,
                    ins=[vb.opt()], outs=[vall.opt()])
                nc.gpsimd.collective_compute(
                    "AllGather", mybir.AluOpType.bypass, replica_groups=groups,
                    ins=[kb2.opt()], outs=[kall2.opt()])

                # gather-ins on the SP queue: the admit-stalls sit behind all
                # early-phase SP traffic and ahead of only norm shifts/out.
                kt_all = kvp.tile([128, G * ET * TOK], FP8, tag="kv")
                v_all = kvp.tile([128, kvslot], FP8, tag="kv")
                for cc in range(G):
                    nc.sync.dma_start(
                        kt_all[:, cc * ET * TOK: cc * ET * TOK + KH],
                        kall1[cc * 128:(cc + 1) * 128, :])
                for cc in range(G):
                    nc.sync.dma_start(
                        v_all[:, cc * VW:(cc + 1) * VW],
                        vall[cc * 128:(cc + 1) * 128, :])
                for cc in range(G):
                    nc.sync.dma_start(
                        kt_all[:, cc * ET * TOK + KH: (cc + 1) * ET * TOK],
                        kall2[cc * 128:(cc + 1) * 128, :])

                # ---- Q ----
                q_sb = loc8.tile([128, ET * TOK], FP8, tag="loc")
                for m in range(ET):
                    qk_block(q_sb, m, wq_d, 0, wqbc)

            # ================= attention =================
            attn_sb = acts.tile([128, ET * TOK], BF16, tag="act8")

            def kt_slice(h, kt):
                cc, l = divmod(kt, TOKT)
                base = (h % 2) * 64
                off = (cc * ET + h // 2) * TOK + l * 128
                return kt_all[base:base + 64, off:off + 128]

            def q_slice(h):
                base = (h % 2) * 64
                return q_sb[base:base + 64, (h // 2) * TOK:(h // 2 + 1) * TOK]

            def v_slice(h, kt):
                off = kt * H * HD1 + h * HD1
                return v_all[:, off:off + HD1]

            with (
                tc.tile_pool(name="sps", bufs=2, space="PSUM") as sps,
                tc.tile_pool(name="avps", bufs=3, space="PSUM") as avps,
                tc.tile_pool(name="bcps", bufs=1, space="PSUM") as bcps,
            ):
                avs = {}
                etile = {}

                def emit_scores(h, g2):
                    s = sps.tile([128, 2 * TOK], F32, tag="s")
                    for j in range(2):
                        kt = 2 * g2 + j
                        nc.tensor.matmul(s[:, j * TOK:(j + 1) * TOK],
                                         kt_slice(h, kt), q_slice(h),
                                         start=True, stop=True)
                    return s

                def emit_exp(h, g2, s):
                    e = expp.tile([128, 2 * TOK], FP8, tag="e")
                    nc.scalar.activation(e[:], s[:], Exp, scale=sm_scale)
                    etile[(h, g2)] = e

                def emit_av(h, g2):
                    if g2 == 0:
                        avs[h] = avps.tile([HD1, TOK], F32, tag="av",
                                           name=f"av_h{h}")
                    av = avs[h]
                    e = etile.pop((h, g2))
                    for j in range(2):
                        kt = 2 * g2 + j
                        nc.tensor.matmul(av[:], v_slice(h, kt),
                                         e[:, j * TOK:(j + 1) * TOK],
                                         start=(kt == 0), stop=(kt == KT - 1))

                def emit_norm(h):
                    # normalize by the gathered denominator (row DH of av psum).
                    # custom-DVE ops misbehave at partition base 64 on HW, so
                    # DMA-shift the denominator row to partition 0 first.
                    av = avs.pop(h)
                    hp = h // 2
                    den64 = rr.tile([128, TOK], F32, tag="den64")
                    nc.vector.tensor_copy(den64[DH:DH + 1, :], av[DH:DH + 1, :])
                    den0 = rr.tile([1, TOK], F32, tag="den0")
                    nc.sync.dma_start(den0[:], den64[DH:DH + 1, :])
                    rrec = rr.tile([1, TOK], F32, tag="rrec")
                    rrecb = rr.tile([1, TOK], BF16, tag="rrecb")
                    nc.vector.reciprocal_approx_fast(rrec[:], den0[:])
                    nc.vector.tensor_copy(rrecb[:], rrec[:])
                    psr = bcps.tile([128, TOK], F32, tag="psr")
                    nc.tensor.matmul(psr[:], ones_full[0:1, :],
                                     rrecb[:], start=True, stop=True)
                    avsb = shp.tile([64, TOK], F32, tag="avsb")
                    nc.vector.tensor_copy(avsb[:], av[0:DH, :])
                    if h % 2 == 0:
                        nc.vector.tensor_mul(
                            attn_sb[0:DH, hp * TOK:(hp + 1) * TOK],
                            avsb[:], psr[0:DH, :])
                    else:
                        tmp = shp.tile([64, TOK], BF16, tag="shift")
                        nc.vector.tensor_mul(tmp[:], avsb[:], psr[0:DH, :])
                        nc.sync.dma_start(
                            attn_sb[64:128, hp * TOK:(hp + 1) * TOK], tmp[:])

                prev = None
                for h in range(H):
                    for g2 in range(NG):
                        s = emit_scores(h, g2)
                        if prev is not None:
                            emit_exp(*prev)
                        prev = (h, g2, s)
                        if h >= 4:
                            emit_av(h - 4, g2)
                        if g2 == 5 and h >= 5:
                            emit_norm(h - 5)
                emit_exp(*prev)
                for hh in range(H - 4, H):
                    for g2 in range(NG):
                        emit_av(hh, g2)
                    emit_norm(hh - 1)
                emit_norm(H - 1)

            # ============ proj + residual (+ LN2 stats inline) ============
            y_sb = resid.tile([128, ET * TOK], F32, tag="resid")
            with tc.tile_pool(name="prps", bufs=2, space="PSUM") as prps, \
                 tc.tile_pool(name="lnps2", bufs=1, space="PSUM") as lnps2:
                st2_s = lnps2.tile([1, TOK], F32, tag="st_s")
                st2_q = lnps2.tile([1, TOK], F32, tag="st_q")
                for m in range(ET):
                    wblk = wstr.tile([128, ET * 128], BF16, tag="wa")
                    nc.sync.dma_start(wblk[:],
                                      wproj_d[:, m * ET * 128:(m + 1) * ET * 128])
                    ps = prps.tile([128, TOK], F32, tag="mm")
                    for k in range(ET):
                        nc.tensor.matmul(ps[:], wblk[:, k * 128:(k + 1) * 128],
                                         attn_sb[:, k * TOK:(k + 1) * TOK],
                                         start=(k == 0), stop=(k == ET - 1))
                    sl = slice(m * TOK, (m + 1) * TOK)
                    nc.vector.scalar_tensor_tensor(
                        out=y_sb[:, sl], in0=ps[:],
                        scalar=bprojc[:, m:m + 1],
                        in1=x_sb[:, sl], op0=add, op1=add)
                    sq = scr.tile([128, TOK], BF16, tag="lnsq", bufs=2)
                    nc.vector.tensor_mul(sq[:], y_sb[:, sl], y_sb[:, sl])
                    nc.tensor.matmul(st2_s[:], ones_col_f32, y_sb[:, sl],
                                     start=(m == 0), stop=(m == ET - 1))
                    nc.tensor.matmul(st2_q[:], ones_col_bf, sq[:],
                                     start=(m == 0), stop=(m == ET - 1))

                # ---- LN2 rows + normalize (g2 folded into broadcasts) ----
                h2_bf = acts.tile([128, ET * TOK], BF16, tag="act8")
                r_mean = rows.tile([1, TOK], F32, tag="rowf")
                r_m2 = rows.tile([1, TOK], F32, tag="rowf2")
                r_msq = rows.tile([1, TOK], F32, tag="rowf3")
                r_var = rows.tile([1, TOK], F32, tag="rowf2b")
                r_rec = rows.tile([1, TOK], F32, tag="rowf3b")
                r_rstd = rows.tile([1, TOK], F32, tag="rowf4")
                r_nmr = rows.tile([1, TOK], F32, tag="rowf5")
                nc.vector.tensor_scalar_mul(r_mean[:], st2_s[:], 1.0 / E)
                nc.vector.tensor_scalar_mul(r_m2[:], st2_q[:], 1.0 / E)
                nc.vector.tensor_mul(r_msq[:], r_mean[:], r_mean[:])
                nc.vector.tensor_sub(r_var[:], r_m2[:], r_msq[:])
                nc.vector.tensor_scalar_add(r_var[:], r_var[:], eps)
                nc.vector.reciprocal_approx_fast(r_rec[:], r_var[:])
                nc.scalar.sqrt(r_rstd[:], r_rec[:])
                nc.vector.scalar_tensor_tensor(
                    out=r_nmr[:], in0=r_mean[:], scalar=-1.0, in1=r_rstd[:],
                    op0=mult, op1=mult)
                r_rstd_bf = rows.tile([1, TOK], BF16, tag="rowbf")
                r_nmr_bf = rows.tile([1, TOK], BF16, tag="rowbf2")
                nc.vector.tensor_copy(r_rstd_bf[:], r_rstd[:])
                nc.vector.tensor_copy(r_nmr_bf[:], r_nmr[:])
                with tc.tile_pool(name="lnbc", bufs=2, space="PSUM") as lnbc:
                    for k in range(ET):
                        sl = slice(k * TOK, (k + 1) * TOK)
                        gk = grow[0:1, k * 128:(k + 1) * 128]
                        ps_rg = lnbc.tile([128, TOK], F32, tag="bc_rg")
                        ps_ng = lnbc.tile([128, TOK], F32, tag="bc_ng")
                        nc.tensor.matmul(ps_rg[:], gk, r_rstd_bf[:],
                                         start=True, stop=True)
                        nc.tensor.matmul(ps_ng[:], gk, r_nmr_bf[:],
                                         start=True, stop=True)
                        t1 = scr.tile([128, TOK], F32, tag="ln_t1", bufs=2)
                        nc.vector.tensor_mul(t1[:], y_sb[:, sl], ps_rg[:])
                        nc.vector.scalar_tensor_tensor(
                            out=h2_bf[:, sl], in0=t1[:],
                            scalar=be2c[:, k:k + 1],
                            op0=add, in1=ps_ng[:], op1=add)

            # ================= FFN =================
            FH = FT // 2 * TOK
            f_a = kvp.tile([128, FH], BF16, tag="kv")
            f_b = kvp.tile([128, FH], BF16, tag="kv")

            def f_sl(m):
                t = f_a if m < FT // 2 else f_b
                mm = m % (FT // 2)
                return t[:, mm * TOK:(mm + 1) * TOK]

            with tc.tile_pool(name="f1ps", bufs=3, space="PSUM") as f1ps:
                for m in range(FT):
                    wblk = wstr.tile([128, ET * 128], BF16, tag="wa")
                    nc.scalar.dma_start(wblk[:],
                                        w1_d[:, m * ET * 128:(m + 1) * ET * 128])
                    ps = f1ps.tile([128, TOK], F32, tag="mm")
                    for k in range(ET):
                        nc.tensor.matmul(ps[:], wblk[:, k * 128:(k + 1) * 128],
                                         h2_bf[:, k * TOK:(k + 1) * TOK],
                                         start=(k == 0), stop=(k == ET - 1))
                    nc.vector.scalar_tensor_tensor(
                        out=f_sl(m), in0=ps[:],
                        scalar=b1c[:, m:m + 1], in1=zeros_bc, op0=add, op1=mx)

            out_sb = resid.tile([128, ET * TOK], F32, tag="resid")
            with tc.tile_pool(name="f2ps", bufs=3, space="PSUM") as f2ps:
                for m in range(ET):
                    wblk2 = wstr2.tile([128, FT * 128], BF16, tag="wb")
                    nc.scalar.dma_start(wblk2[:],
                                        w2_d[:, m * FT * 128:(m + 1) * FT * 128])
                    ps = f2ps.tile([128, TOK], F32, tag="mm")
                    for k in range(FT):
                        nc.tensor.matmul(ps[:], wblk2[:, k * 128:(k + 1) * 128],
                                         f_sl(k),
                                         start=(k == 0), stop=(k == FT - 1))
                    nc.vector.scalar_tensor_tensor(
                        out=out_sb[:, m * TOK:(m + 1) * TOK], in0=ps[:],
                        scalar=b2c[:, m:m + 1],
                        in1=y_sb[:, m * TOK:(m + 1) * TOK], op0=add, op1=add)
                    nc.sync.dma_start(out_d[:, m * TOK:(m + 1) * TOK],
                                      out_sb[:, m * TOK:(m + 1) * TOK])

    nc.compile()
    return nc


# ---------------- host-side packing ----------------

def _colblk(w2d, kt, mt):
    return np.ascontiguousarray(
        w2d.reshape(kt, 128, mt, 128).transpose(1, 2, 0, 3).reshape(128, mt * kt * 128))


def _rowmaj(w2d, kt):
    n = w2d.shape[1]
    return np.ascontiguousarray(
        w2d.reshape(kt, 128, n).transpose(1, 0, 2).reshape(128, kt * n))


def _fm(chunk_te, et, tok):
    return np.ascontiguousarray(
        chunk_te.T.reshape(et, 128, tok).transpose(1, 0, 2).reshape(128, et * tok))


def _cols(v, t):
    return np.ascontiguousarray(v.reshape(t, 128).T)


def make_in_maps(dims, x, Wq, Wk, Wv, Wproj, bproj, W1, b1, W2, b2,
                 g1, beta1, g2, beta2):
    E, H, DH, TOK, G, NC, FF = (
        dims["E"], dims["H"], dims["DH"], dims["TOK"], dims["G"], dims["NC"], dims["FF"]
    )
    ET, FT = E // 128, FF // 128
    bf = ml_dtypes.bfloat16
    wq2 = Wq.transpose(1, 0, 2).reshape(E, H * DH)
    wk2 = Wk.transpose(1, 0, 2).reshape(E, H * DH)
    wv2 = Wv.transpose(1, 0, 2).reshape(E, H * DH)
    wqkvg = np.concatenate([w.T @ g1 for w in (wq2, wk2, wv2)]).reshape(1, -1)
    wvbr = (wv2.T @ beta1).reshape(1, -1)
    shared = {
        "wq": _colblk(wq2, ET, (H * DH) // 128).astype(bf),
        "wk": _colblk(wk2, ET, (H * DH) // 128).astype(bf),
        "wv": _rowmaj(wv2, ET).astype(bf),
        "wproj": _colblk(Wproj, (H * DH) // 128, ET).astype(bf),
        "w1": _colblk(W1, ET, FT).astype(bf),
        "w2": _colblk(W2, FT, ET).astype(bf),
        "wqkvg": np.ascontiguousarray(wqkvg).astype(bf),
        "wvbr": np.ascontiguousarray(wvbr).astype(bf),
        "g1c": _cols(g1, ET).astype(np.float32),
        "g2r": np.ascontiguousarray(g2.reshape(1, E)).astype(bf),
        "wqbc": _cols(wq2.T @ beta1, ET).astype(np.float32),
        "wkbc": _cols(wk2.T @ beta1, ET).astype(np.float32),
        "be2c": _cols(beta2, ET).astype(np.float32),
        "bprojc": _cols(bproj, ET).astype(np.float32),
        "b1c": _cols(b1, FT).astype(np.float32),
        "b2c": _cols(b2, ET).astype(np.float32),
    }
    xf = x.reshape(-1, E)
    in_maps = []
    for r in range(NC):
        xc = xf[r * TOK:(r + 1) * TOK, :]
        m = dict(shared)
        m["x"] = _fm(xc.astype(np.float32), ET, TOK)
        in_maps.append(m)
    return in_maps


def assemble_out(dims, results):
    E, TOK, NC = dims["E"], dims["TOK"], dims["NC"]
    ET = E // 128
    outs = []
    for r in range(NC):
        o = results[r]["outT"]
        outs.append(o.reshape(128, ET, TOK).transpose(1, 0, 2).reshape(E, TOK).T)
    return np.concatenate(outs, axis=0)


_NC_CACHE = {}


def kernel(x, Wq, Wk, Wv, Wproj, bproj, W1, b1, W2, b2, g1, beta1, g2, beta2,
           **extra):
    dims = FULL_DIMS
    arrs = dict(x=np.asarray(x, np.float32))
    for k, v in dict(Wq=Wq, Wk=Wk, Wv=Wv, Wproj=Wproj, bproj=bproj, W1=W1,
                     b1=b1, W2=W2, b2=b2, g1=g1, beta1=beta1, g2=g2,
                     beta2=beta2).items():
        arrs[k] = np.asarray(v, np.float32)
    in_maps = make_in_maps(dims, **arrs)
    key = "full"
    if key not in _NC_CACHE:
        _NC_CACHE[key] = build_nc(dims)
    nc = _NC_CACHE[key]
    res = run_bass_kernel_spmd(nc, in_maps, core_ids=list(range(dims["NC"])))
    flat = assemble_out(dims, res.results)
    B = x.shape[0]
    return flat.reshape(B, -1, dims["E"]).astype(np.float32)
